# revision 15
# baseline (speedup 1.0000x reference)
"""BinaryTreeRNN forward pass on 8 Trainium2 NeuronCores.

Strategy (pure data parallel, per the sharding hint):
  - Shard x row-wise into 8 shards; replicate the ~100 tree parameters
    (folded into matmul weights + per-op float immediates on host).
  - Host pre-transposes x into the device layout [128, NPAD/8] fp16
    (partition = 16*j + v for 8 interleaved samples x 16 vars), so input
    DMA is plain contiguous at full HBM rate.
  - Per core: a block-diagonal [128, 64] fp16 stationary computes all 8
    leaves for 8 interleaved samples per PE column; leaves get bias during
    the PSUM->SBUF cast (ScalarE Identity + per-partition bias vector),
    then TensorE transposes put samples on partitions (PSUM fp16).
  - 3-level tree reduction, work split across engines:
      s = l + r (DVE TT 2x), p = l * r (DVE TT 2x)
      u = A*s + D (DVE TS 4x, per-node slices)
      p' = B*p (DVE TS 4x), h = u + p' (DVE TT)
      range-reduce: k = round(s*g/2pi) (DVE TS, int cast rounds),
        kf = -2pi/g*k (DVE TS 4x), r = s + kf (DVE TT 2x)
      q = sin(g*r)  (ScalarE); q' = C*q (DVE TS 4x)
      h += q' on Pool (TT); everything else on DVE
  - Power-of-two per-level scales keep fp16 intermediates in range.
  - Output written contiguously per device order, un-permuted on host.
"""
import os
import sys

sys.path.insert(0, "/opt/trn_rl_repo")

import numpy as np

import concourse.bass as bass
import concourse.mybir as mybir
import concourse.tile as tile
from concourse.bass_utils import run_bass_kernel_spmd

F16 = mybir.dt.float16
F32 = mybir.dt.float32
I16 = mybir.dt.int16
I32 = mybir.dt.int32

N_CORES = 8
N_TOTAL = 2_000_000
SHARD = N_TOTAL // N_CORES          # 250_000
TREE_GROUPS = [[8192, 16384], [65536, 65536, 65536], [32768]]
BLOCKS = [b for tg in TREE_GROUPS for b in tg]
NPAD = sum(BLOCKS)                  # 253_952
TWO_PI = float(2.0 * np.pi)

# leaf permutation: v' 0..3 = left children (leaves 0,2,4,6), 4..7 = right
PERM = np.array([0, 2, 4, 6, 1, 3, 5, 7])


def _sm(om):
    e = np.exp(om - om.max(axis=-1, keepdims=True))
    return e / e.sum(axis=-1, keepdims=True)


def _pow2_at_least(x):
    """Smallest power of two >= max(x, 1)."""
    return float(2.0 ** np.ceil(np.log2(max(float(x), 1.0))))


def _fold_params(inputs, xmax):
    """Fold tree parameters into device constants + per-op immediates."""
    W = np.asarray(inputs["W_leaf"], np.float64)
    bl = np.asarray(inputs["b_leaf"], np.float64)
    lv = {}
    for lev, nn in ((0, 1), (1, 2), (2, 4)):
        w = np.asarray(inputs[f"w{lev}"], np.float64)
        b = np.asarray(inputs[f"b{lev}"], np.float64)
        sm = _sm(np.asarray(inputs[f"om{lev}"], np.float64))
        lv[lev] = dict(
            A=w * (sm[:, 0] + sm[:, 3]),
            B=w * sm[:, 1],
            C=w * sm[:, 2],
            D=b,
        )

    # fp16 weights as actually used on device
    W16 = W[PERM].astype(np.float16).astype(np.float64)       # [8, 16] (perm order)
    bl16 = bl[PERM]                                            # bias kept fp32

    # interval bounds (true magnitudes)
    lb = (np.abs(W16).sum(axis=1) * xmax + np.abs(bl16)) * 1.05 + 1e-6  # [8]
    g_leaf = 1.0
    if lb.max() > 200.0:
        g_leaf = _pow2_at_least(lb.max() / 200.0)

    s2b = lb[0:4] + lb[4:8]
    p2b = lb[0:4] * lb[4:8]
    h2b = (np.abs(lv[2]["A"]) * s2b + np.abs(lv[2]["B"]) * p2b
           + np.abs(lv[2]["C"]) + np.abs(lv[2]["D"])) * 1.05 + 1e-6
    g2 = _pow2_at_least(h2b.max() / 200.0)

    s1b = h2b[0::2] + h2b[1::2]
    p1b = h2b[0::2] * h2b[1::2]
    h1b = (np.abs(lv[1]["A"]) * s1b + np.abs(lv[1]["B"]) * p1b
           + np.abs(lv[1]["C"]) + np.abs(lv[1]["D"])) * 1.05 + 1e-6
    g1 = _pow2_at_least(h1b.max() / 200.0)

    s0b = h1b[0] + h1b[1]
    p0b = h1b[0] * h1b[1]
    h0b = float(((np.abs(lv[0]["A"]) * s0b + np.abs(lv[0]["B"]) * p0b
                  + np.abs(lv[0]["C"]) + np.abs(lv[0]["D"])) * 1.05 + 1e-6)[0])
    g0 = _pow2_at_least(h0b / 40000.0)

    # blockdiag stationary G [128, 64] (leaf scale folded in)
    G = np.zeros((128, 64), np.float32)
    for j in range(8):
        for vp in range(8):
            G[16 * j:16 * j + 16, 8 * vp + j] = (W16[vp] / g_leaf).astype(np.float32)
    biasvec = np.zeros((128, 1), np.float32)
    for h in range(2):
        for vp in range(8):
            for j in range(8):
                biasvec[64 * h + 8 * vp + j, 0] = bl16[vp] / g_leaf

    # per-level op immediates: children scale gp -> own scale gc
    s_bounds = {2: float(s2b.max()), 1: float(s1b.max()), 0: float(s0b)}

    def imm(lev, gp, gc):
        d = lv[lev]
        kmax = s_bounds[lev] / (2.0 * np.pi)   # |k| bound (true periods)
        return dict(
            ts1=[float(a * gp / gc) for a in d["A"]],
            ts2=[float(dd / gc) for dd in d["D"]],
            sp=[float(b * gp * gp / gc) for b in d["B"]],
            sq=[float(c / gc) for c in d["C"]],
            k_scale=float(gp / TWO_PI),
            kf_scale=float(-TWO_PI / gp),
            sin_scale=float(gp),
            k_i32=bool(kmax > 30000.0),
            need_rr=bool(s_bounds[lev] > 3.0),
        )

    return dict(
        G=G.astype(np.float16),
        biasvec=biasvec,
        L2=imm(2, g_leaf, g2),
        L1=imm(1, g2, g1),
        L0=imm(0, g1, g0),
        g0=float(g0),
    )


# ---------------------------------------------------------------------------
# walrus in this container accepts at most ONE sync-wait per instruction
# (2 for InstEventSemaphore); hoist excess waits onto InstNoOp carriers.
def _split_excess_waits(nc):
    n_fix = 0
    for fn in nc.m.functions:
        for blk in fn.blocks:
            new_insts = []
            for inst in blk.instructions:
                si = inst.sync_info
                cap = 2 if isinstance(inst, mybir.InstEventSemaphore) else 1
                if si is not None and len(si.on_wait) > cap:
                    waits = list(si.on_wait)
                    for w in waits[:-cap]:
                        new_insts.append(mybir.InstNoOp(
                            name=f"{inst.name}-waitc{n_fix}",
                            ins=[], outs=[],
                            sync_info=mybir.SyncInfo(on_wait=[w], on_update=[]),
                            bass_nofuse=True,
                            engine=inst.engine,
                        ))
                        n_fix += 1
                    inst.sync_info = mybir.SyncInfo(
                        on_wait=waits[-cap:], on_update=list(si.on_update))
                new_insts.append(inst)
            blk.instructions[:] = new_insts
    return n_fix


def _build_program(cc):
    """cc: folded constants (for the float immediates)."""
    nc = bass.Bass("TRN2", target_bir_lowering=False, debug=False,
                   num_devices=N_CORES)
    x_d = nc.dram_tensor("x", [128, NPAD // 8], F16, kind="ExternalInput").ap()
    c16_d = nc.dram_tensor("c16", [128, 192], F16, kind="ExternalInput").ap()
    bv_d = nc.dram_tensor("bv", [128, 1], F32, kind="ExternalInput").ap()
    y_d = nc.dram_tensor("y", [128, NPAD // 128], F16, kind="ExternalOutput").ap()

    Sin = mybir.ActivationFunctionType.Sin
    Ident = mybir.ActivationFunctionType.Identity
    MUL = mybir.AluOpType.mult
    ADD = mybir.AluOpType.add
    MOD = mybir.AluOpType.mod

    with tile.TileContext(nc) as tc:
        with tc.tile_pool(name="cpool", bufs=1) as cpool, \
             tc.tile_pool(name="xpool", bufs=2) as xpool, \
             tc.tile_pool(name="vpool", bufs=4) as vpool, \
             tc.tile_pool(name="tpool", bufs=1) as tpool, \
             tc.tile_pool(name="ypool", bufs=2) as ypool, \
             tc.tile_pool(name="psum", bufs=2, space="PSUM") as ppool, \
             tc.tile_pool(name="psumL", bufs=2, space="PSUM") as lpool:

            c16 = cpool.tile([128, 192], F16)
            bvt = cpool.tile([128, 1], F32)
            with tc.high_priority():
                nc.sync.dma_start(out=c16[:], in_=c16_d[:])
                nc.sync.dma_start(out=bvt[:], in_=bv_d[:])
            Gt = c16[:, 0:64]
            idt = c16[:, 64:192]

            def split_tt(out, a, b, op, nf, pool_frac=0.34):
                """TT with the free range split DVE/Pool so both engines run
                in parallel on disjoint slices (dim 1 = NF chunks)."""
                k = max(1, min(nf - 1, int(round(nf * (1.0 - pool_frac)))))
                nc.vector.tensor_tensor(out[:, 0:k], a[:, 0:k], b[:, 0:k], op)
                nc.gpsimd.tensor_tensor(out[:, k:nf], a[:, k:nf], b[:, k:nf],
                                        op)

            def rr_sin(S, Kt, KF, Q, imm, nf):
                """Q = sin(gp*s): k = round(s*gp/2pi) (DVE int cast rounds),
                kf = -2pi/gp * k, r = s + kf, sin(gp*r) on ScalarE."""
                if imm["need_rr"]:
                    nc.vector.tensor_scalar(Kt[:], S[:], imm["k_scale"],
                                            None, MUL)
                    nc.vector.tensor_scalar(KF[:], Kt[:], imm["kf_scale"],
                                            None, MUL)
                    split_tt(KF, S, KF, ADD, nf)
                    nc.scalar.activation(Q[:], KF[:], Sin,
                                         scale=imm["sin_scale"])
                else:
                    nc.scalar.activation(Q[:], S[:], Sin,
                                         scale=imm["sin_scale"])

            row0 = 0
            for bi, TG in enumerate(TREE_GROUPS):
                TB = sum(TG)            # samples in this tree group
                NF = TB // 2048         # 2048-sample leaf chunks
                trow0 = row0

                # leaves (biased, fp16) land in PSUM via PE transpose;
                # matmul blocks stay fine-grained for pipelining while the
                # tree tiles span the whole group (fewer, bigger DVE ops).
                LL = tpool.tile([128, NF, 2, 32], F16, name=f"LL_{bi}", tag="LL")
                S2 = tpool.tile([128, NF, 2, 32], F16, name=f"S2_{bi}", tag="S2")
                PB2 = tpool.tile([128, NF, 2, 32], F16, name=f"PB2_{bi}", tag="PB2")
                nfb = 0
                for bj, B in enumerate(TG):
                    R = B // 8              # xT columns
                    NP = B // 8192          # matmul pairs

                    xT = xpool.tile([128, R], F16, name=f"xT{bi}_{bj}",
                                    tag="xT")
                    half = R // 2
                    nc.sync.dma_start(out=xT[:, 0:half],
                                      in_=x_d[:, row0:row0 + half])
                    nc.sync.dma_start(out=xT[:, half:R],
                                      in_=x_d[:, row0 + half:row0 + R])

                    nq = (NP + 3) // 4
                    quarters = [(4 * qi, min(4 * qi + 4, NP))
                                for qi in range(nq)]
                    for hi, (p0, p1) in enumerate(quarters):
                        nfh = 4 * (p1 - p0)
                        leafT = lpool.tile([128, nfh, 128], F16,
                                           name=f"leafT{bi}_{bj}_{hi}",
                                           tag="leafT")
                        groups = [(c0, 2) for c0 in range(p0, p1 - 1, 2)]
                        if (p1 - p0) % 2:
                            groups.append((p1 - 1, 1))
                        for gi, (c0, ng) in enumerate(groups):
                            vps = ppool.tile([128, 1024], F32,
                                             name=f"vps{bi}_{bj}_{hi}_{gi}",
                                             tag="vps")
                            vt = vpool.tile([128, 1024], F16,
                                            name=f"vt{bi}_{bj}_{hi}_{gi}",
                                            tag="vt")
                            for q in range(2 * ng):
                                nc.tensor.matmul(
                                    vps[64 * (q % 2):64 * (q % 2) + 64,
                                        512 * (q // 2):512 * (q // 2) + 512],
                                    Gt,
                                    xT[:, 1024 * c0 + 512 * q:
                                       1024 * c0 + 512 * q + 512],
                                    start=True, stop=True)
                            nc.scalar.activation(vt[:, 0:512 * ng],
                                                 vps[:, 0:512 * ng],
                                                 Ident, bias=bvt[:, 0:1])
                            for u in range(4 * ng):
                                nc.tensor.transpose(
                                    leafT[:, 4 * (c0 - p0) + u, :],
                                    vt[:, 128 * u:128 * (u + 1)],
                                    idt)
                        lvh = leafT.rearrange("p n (h w) -> p n h w", h=2)
                        nf0 = nfb + 4 * p0
                        hsl = slice(nf0, nf0 + nfh)
                        nc.scalar.activation(LL[:, hsl], lvh[:, :, :, 0:32],
                                             Ident)
                        nc.vector.tensor_tensor(S2[:, hsl], LL[:, hsl],
                                                lvh[:, :, :, 32:64], ADD)
                        nc.vector.tensor_tensor(PB2[:, hsl], LL[:, hsl],
                                                lvh[:, :, :, 32:64], MUL)
                    nfb += B // 2048
                    row0 += R

                # ---- L2 ----
                imm = cc["L2"]
                Q2 = tpool.tile([128, NF, 2, 32], F16, name=f"Q2_{bi}", tag="Q2")
                K2 = tpool.tile([128, NF, 2, 32],
                                I32 if imm["k_i32"] else I16,
                                name=f"K2_{bi}", tag="K2")
                KF2 = tpool.tile([128, NF, 2, 32], F16, name=f"KF2_{bi}", tag="KF2")
                H2 = tpool.tile([128, NF, 2, 32], F16, name=f"H2_{bi}", tag="H2")
                rr_sin(S2, K2, KF2, Q2, imm, NF)
                for k in range(4):
                    sl = (slice(None), slice(None), slice(None),
                          slice(8 * k, 8 * k + 8))
                    nc.vector.tensor_scalar(H2[sl], S2[sl], imm["ts1"][k],
                                            imm["ts2"][k], MUL, ADD)
                    nc.vector.tensor_scalar(PB2[sl], PB2[sl],
                                            imm["sp"][k], None, MUL)
                    nc.vector.tensor_scalar(Q2[sl], Q2[sl], imm["sq"][k],
                                            None, MUL)
                split_tt(H2, H2, PB2, ADD, NF)
                split_tt(H2, H2, Q2, ADD, NF)

                # ---- L1 ----  (children at z 0:8 / 8:16 of kp groups)
                imm = cc["L1"]
                h2q = H2.rearrange("p n h (kp z) -> p (n h) kp z", kp=2)
                l1, r1 = h2q[:, :, :, 0:8], h2q[:, :, :, 8:16]
                S1 = tpool.tile([128, 2 * NF, 2, 8], F16, name=f"S1_{bi}", tag="S1")
                PB1 = tpool.tile([128, 2 * NF, 2, 8], F16, name=f"PB1_{bi}", tag="PB1")
                Q1 = tpool.tile([128, 2 * NF, 2, 8], F16, name=f"Q1_{bi}", tag="Q1")
                K1 = tpool.tile([128, 2 * NF, 2, 8],
                                I32 if imm["k_i32"] else I16,
                                name=f"K1_{bi}", tag="K1")
                KF1 = tpool.tile([128, 2 * NF, 2, 8], F16, name=f"KF1_{bi}", tag="KF1")
                H1 = tpool.tile([128, 2 * NF, 2, 8], F16, name=f"H1_{bi}", tag="H1")
                nc.vector.tensor_tensor(S1[:], l1, r1, ADD)
                nc.vector.tensor_tensor(PB1[:], l1, r1, MUL)
                rr_sin(S1, K1, KF1, Q1, imm, 2 * NF)
                for m in range(2):
                    sl = (slice(None), slice(None), slice(m, m + 1), slice(None))
                    nc.vector.tensor_scalar(H1[sl], S1[sl], imm["ts1"][m],
                                            imm["ts2"][m], MUL, ADD)
                    nc.vector.tensor_scalar(PB1[sl], PB1[sl],
                                            imm["sp"][m], None, MUL)
                for m in range(2):
                    sl = (slice(None), slice(None), slice(m, m + 1), slice(None))
                    nc.vector.tensor_scalar(Q1[sl], Q1[sl], imm["sq"][m],
                                            None, MUL)
                split_tt(H1, H1, PB1, ADD, 2 * NF)
                split_tt(H1, H1, Q1, ADD, 2 * NF)

                # ---- L0 ----
                imm = cc["L0"]
                S0 = tpool.tile([128, 2 * NF, 8], F16, name=f"S0_{bi}", tag="S0")
                PB0 = tpool.tile([128, 2 * NF, 8], F16, name=f"PB0_{bi}", tag="PB0")
                Q0 = tpool.tile([128, 2 * NF, 8], F16, name=f"Q0_{bi}", tag="Q0")
                K0 = tpool.tile([128, 2 * NF, 8],
                                I32 if imm["k_i32"] else I16,
                                name=f"K0_{bi}", tag="K0")
                KF0 = tpool.tile([128, 2 * NF, 8], F16, name=f"KF0_{bi}", tag="KF0")
                Y = ypool.tile([128, 2 * NF, 8], F16, name=f"Y_{bi}", tag="Y")
                nc.vector.tensor_tensor(S0[:], H1[:, :, 0:1, :], H1[:, :, 1:2, :], ADD)
                nc.vector.tensor_tensor(PB0[:], H1[:, :, 0:1, :],
                                        H1[:, :, 1:2, :], MUL)
                rr_sin(S0, K0, KF0, Q0, imm, 2 * NF)
                nc.vector.tensor_scalar(Y[:], S0[:], imm["ts1"][0],
                                        imm["ts2"][0], MUL, ADD)
                nc.vector.tensor_scalar(PB0[:], PB0[:], imm["sp"][0],
                                        None, MUL)
                nc.vector.tensor_scalar(Q0[:], Q0[:], imm["sq"][0], None, MUL)
                split_tt(Y, Y, PB0, ADD, 2 * NF)
                split_tt(Y, Y, Q0, ADD, 2 * NF)

                nc.sync.dma_start(out=y_d[:, trow0 // 16:
                                          trow0 // 16 + TB // 128],
                                  in_=Y[:])

    _split_excess_waits(nc)
    return nc


def _unpermute(y_core):
    """y_core [128, NPAD//128] fp16 -> [NPAD] f32 in sample order."""
    out = np.empty(NPAD, np.float32)
    base = 0
    col0 = 0
    for TG in TREE_GROUPS:
        TB = sum(TG)
        NP = TB // 8192
        FD = TB // 128
        yb = y_core[:, col0:col0 + FD].astype(np.float32)
        y5 = yb.reshape(128, NP, 4, 2, 8)          # q, c, u, h, j
        out[base:base + TB] = y5.transpose(1, 3, 2, 0, 4).reshape(TB)
        base += TB
        col0 += FD
    return out


def kernel(**inputs):
    x = np.asarray(inputs["x"], np.float32)
    xmax = float(np.abs(x).max())
    cc = _fold_params(inputs, xmax)

    nc = _build_program(cc)

    xp = np.zeros((N_CORES, NPAD, 16), np.float16)
    xp[:, :SHARD] = x.reshape(N_CORES, SHARD, 16).astype(np.float16)
    # [cores, 128, NPAD//8]: device layout xT[16j+v, r] = x[8r+j, v]
    xt = np.ascontiguousarray(
        xp.reshape(N_CORES, NPAD // 8, 128).transpose(0, 2, 1))

    c16 = np.concatenate([cc["G"], np.eye(128, dtype=np.float16)], axis=1)
    in_maps = [{"x": xt[c], "c16": c16, "bv": cc["biasvec"]}
               for c in range(N_CORES)]

    trace = bool(os.environ.get("BTREE_TRACE"))
    if trace:
        try:
            res = run_bass_kernel_spmd(nc, in_maps,
                                       core_ids=list(range(N_CORES)),
                                       trace=True)
        except Exception as e:
            print(f"trace run failed ({type(e).__name__}: {e}); rerunning untraced")
            res = run_bass_kernel_spmd(nc, in_maps,
                                       core_ids=list(range(N_CORES)))
    else:
        res = run_bass_kernel_spmd(nc, in_maps, core_ids=list(range(N_CORES)))
    globals()["LAST_RESULTS"] = res

    out = np.empty(N_TOTAL, np.float32)
    for c in range(N_CORES):
        yc = _unpermute(res.results[c]["y"])
        out[c * SHARD:(c + 1) * SHARD] = yc[:SHARD] * cc["g0"]
    return out


# revision 16
# speedup vs baseline: 1.0440x; 1.0440x over previous
"""BinaryTreeRNN forward pass on 8 Trainium2 NeuronCores.

Strategy (pure data parallel, per the sharding hint):
  - Shard x row-wise into 8 shards; replicate the ~100 tree parameters
    (folded into matmul weights + per-op float immediates on host).
  - Host pre-transposes x into the device layout [128, NPAD/8] fp16
    (partition = 16*j + v for 8 interleaved samples x 16 vars), so input
    DMA is plain contiguous at full HBM rate.
  - Per core: a block-diagonal [128, 64] fp16 stationary computes all 8
    leaves for 8 interleaved samples per PE column; leaves get bias during
    the PSUM->SBUF cast (ScalarE Identity + per-partition bias vector),
    then TensorE transposes put samples on partitions (PSUM fp16).
  - 3-level tree reduction, work split across engines:
      s = l + r (DVE TT 2x), p = l * r (DVE TT 2x)
      u = A*s + D (DVE TS 4x, per-node slices)
      p' = B*p (DVE TS 4x), h = u + p' (DVE TT)
      range-reduce: k = round(s*g/2pi) (DVE TS, int cast rounds),
        kf = -2pi/g*k (DVE TS 4x), r = s + kf (DVE TT 2x)
      q = sin(g*r)  (ScalarE); q' = C*q (DVE TS 4x)
      h += q' on Pool (TT); everything else on DVE
  - Power-of-two per-level scales keep fp16 intermediates in range.
  - Output written contiguously per device order, un-permuted on host.
"""
import os
import sys

sys.path.insert(0, "/opt/trn_rl_repo")

import numpy as np

import concourse.bass as bass
import concourse.mybir as mybir
import concourse.tile as tile
from concourse.bass_utils import run_bass_kernel_spmd

F16 = mybir.dt.float16
F32 = mybir.dt.float32
I16 = mybir.dt.int16
I32 = mybir.dt.int32

N_CORES = 8
N_TOTAL = 2_000_000
SHARD = N_TOTAL // N_CORES          # 250_000
TREE_GROUPS = [[8192, 16384], [65536, 16384], [65536, 8192], [65536, 8192]]
BLOCKS = [b for tg in TREE_GROUPS for b in tg]
NPAD = sum(BLOCKS)                  # 253_952
TWO_PI = float(2.0 * np.pi)

# leaf permutation: v' 0..3 = left children (leaves 0,2,4,6), 4..7 = right
PERM = np.array([0, 2, 4, 6, 1, 3, 5, 7])


def _sm(om):
    e = np.exp(om - om.max(axis=-1, keepdims=True))
    return e / e.sum(axis=-1, keepdims=True)


def _pow2_at_least(x):
    """Smallest power of two >= max(x, 1)."""
    return float(2.0 ** np.ceil(np.log2(max(float(x), 1.0))))


def _fold_params(inputs, xmax):
    """Fold tree parameters into device constants + per-op immediates."""
    W = np.asarray(inputs["W_leaf"], np.float64)
    bl = np.asarray(inputs["b_leaf"], np.float64)
    lv = {}
    for lev, nn in ((0, 1), (1, 2), (2, 4)):
        w = np.asarray(inputs[f"w{lev}"], np.float64)
        b = np.asarray(inputs[f"b{lev}"], np.float64)
        sm = _sm(np.asarray(inputs[f"om{lev}"], np.float64))
        lv[lev] = dict(
            A=w * (sm[:, 0] + sm[:, 3]),
            B=w * sm[:, 1],
            C=w * sm[:, 2],
            D=b,
        )

    # fp16 weights as actually used on device
    W16 = W[PERM].astype(np.float16).astype(np.float64)       # [8, 16] (perm order)
    bl16 = bl[PERM]                                            # bias kept fp32

    # interval bounds (true magnitudes)
    lb = (np.abs(W16).sum(axis=1) * xmax + np.abs(bl16)) * 1.05 + 1e-6  # [8]
    g_leaf = 1.0
    if lb.max() > 200.0:
        g_leaf = _pow2_at_least(lb.max() / 200.0)

    s2b = lb[0:4] + lb[4:8]
    p2b = lb[0:4] * lb[4:8]
    h2b = (np.abs(lv[2]["A"]) * s2b + np.abs(lv[2]["B"]) * p2b
           + np.abs(lv[2]["C"]) + np.abs(lv[2]["D"])) * 1.05 + 1e-6
    g2 = _pow2_at_least(h2b.max() / 200.0)

    s1b = h2b[0::2] + h2b[1::2]
    p1b = h2b[0::2] * h2b[1::2]
    h1b = (np.abs(lv[1]["A"]) * s1b + np.abs(lv[1]["B"]) * p1b
           + np.abs(lv[1]["C"]) + np.abs(lv[1]["D"])) * 1.05 + 1e-6
    g1 = _pow2_at_least(h1b.max() / 200.0)

    s0b = h1b[0] + h1b[1]
    p0b = h1b[0] * h1b[1]
    h0b = float(((np.abs(lv[0]["A"]) * s0b + np.abs(lv[0]["B"]) * p0b
                  + np.abs(lv[0]["C"]) + np.abs(lv[0]["D"])) * 1.05 + 1e-6)[0])
    g0 = _pow2_at_least(h0b / 40000.0)

    # blockdiag stationary G [128, 64] (leaf scale folded in)
    G = np.zeros((128, 64), np.float32)
    for j in range(8):
        for vp in range(8):
            G[16 * j:16 * j + 16, 8 * vp + j] = (W16[vp] / g_leaf).astype(np.float32)
    biasvec = np.zeros((128, 1), np.float32)
    for h in range(2):
        for vp in range(8):
            for j in range(8):
                biasvec[64 * h + 8 * vp + j, 0] = bl16[vp] / g_leaf

    # per-level op immediates: children scale gp -> own scale gc
    s_bounds = {2: float(s2b.max()), 1: float(s1b.max()), 0: float(s0b)}

    def imm(lev, gp, gc):
        d = lv[lev]
        kmax = s_bounds[lev] / (2.0 * np.pi)   # |k| bound (true periods)
        return dict(
            ts1=[float(a * gp / gc) for a in d["A"]],
            ts2=[float(dd / gc) for dd in d["D"]],
            sp=[float(b * gp * gp / gc) for b in d["B"]],
            sq=[float(c / gc) for c in d["C"]],
            k_scale=float(gp / TWO_PI),
            kf_scale=float(-TWO_PI / gp),
            sin_scale=float(gp),
            k_i32=bool(kmax > 30000.0),
            need_rr=bool(s_bounds[lev] > 3.0),
        )

    return dict(
        G=G.astype(np.float16),
        biasvec=biasvec,
        L2=imm(2, g_leaf, g2),
        L1=imm(1, g2, g1),
        L0=imm(0, g1, g0),
        g0=float(g0),
    )


# ---------------------------------------------------------------------------
# walrus in this container accepts at most ONE sync-wait per instruction
# (2 for InstEventSemaphore); hoist excess waits onto InstNoOp carriers.
def _split_excess_waits(nc):
    n_fix = 0
    for fn in nc.m.functions:
        for blk in fn.blocks:
            new_insts = []
            for inst in blk.instructions:
                si = inst.sync_info
                cap = 2 if isinstance(inst, mybir.InstEventSemaphore) else 1
                if si is not None and len(si.on_wait) > cap:
                    waits = list(si.on_wait)
                    for w in waits[:-cap]:
                        new_insts.append(mybir.InstNoOp(
                            name=f"{inst.name}-waitc{n_fix}",
                            ins=[], outs=[],
                            sync_info=mybir.SyncInfo(on_wait=[w], on_update=[]),
                            bass_nofuse=True,
                            engine=inst.engine,
                        ))
                        n_fix += 1
                    inst.sync_info = mybir.SyncInfo(
                        on_wait=waits[-cap:], on_update=list(si.on_update))
                new_insts.append(inst)
            blk.instructions[:] = new_insts
    return n_fix


def _build_program(cc):
    """cc: folded constants (for the float immediates)."""
    nc = bass.Bass("TRN2", target_bir_lowering=False, debug=False,
                   num_devices=N_CORES)
    x_d = nc.dram_tensor("x", [128, NPAD // 8], F16, kind="ExternalInput").ap()
    c16_d = nc.dram_tensor("c16", [128, 192], F16, kind="ExternalInput").ap()
    bv_d = nc.dram_tensor("bv", [128, 1], F32, kind="ExternalInput").ap()
    y_d = nc.dram_tensor("y", [128, NPAD // 128], F16, kind="ExternalOutput").ap()

    Sin = mybir.ActivationFunctionType.Sin
    Ident = mybir.ActivationFunctionType.Identity
    MUL = mybir.AluOpType.mult
    ADD = mybir.AluOpType.add
    MOD = mybir.AluOpType.mod

    with tile.TileContext(nc) as tc:
        with tc.tile_pool(name="cpool", bufs=1) as cpool, \
             tc.tile_pool(name="xpool", bufs=2) as xpool, \
             tc.tile_pool(name="vpool", bufs=4) as vpool, \
             tc.tile_pool(name="tpool", bufs=1) as tpool, \
             tc.tile_pool(name="ypool", bufs=2) as ypool, \
             tc.tile_pool(name="psum", bufs=2, space="PSUM") as ppool, \
             tc.tile_pool(name="psumL", bufs=2, space="PSUM") as lpool:

            c16 = cpool.tile([128, 192], F16)
            bvt = cpool.tile([128, 1], F32)
            with tc.high_priority():
                nc.sync.dma_start(out=c16[:], in_=c16_d[:])
                nc.sync.dma_start(out=bvt[:], in_=bv_d[:])
            Gt = c16[:, 0:64]
            idt = c16[:, 64:192]

            def split_tt(out, a, b, op, nf, pool_frac=0.34):
                """TT with the free range split DVE/Pool so both engines run
                in parallel on disjoint slices (dim 1 = NF chunks)."""
                k = max(1, min(nf - 1, int(round(nf * (1.0 - pool_frac)))))
                nc.vector.tensor_tensor(out[:, 0:k], a[:, 0:k], b[:, 0:k], op)
                nc.gpsimd.tensor_tensor(out[:, k:nf], a[:, k:nf], b[:, k:nf],
                                        op)

            def rr_sin(S, Kt, KF, Q, imm, nf):
                """Q = sin(gp*s): k = round(s*gp/2pi) (DVE int cast rounds),
                kf = -2pi/gp * k, r = s + kf, sin(gp*r) on ScalarE."""
                if imm["need_rr"]:
                    nc.vector.tensor_scalar(Kt[:], S[:], imm["k_scale"],
                                            None, MUL)
                    nc.vector.tensor_scalar(KF[:], Kt[:], imm["kf_scale"],
                                            None, MUL)
                    split_tt(KF, S, KF, ADD, nf)
                    nc.scalar.activation(Q[:], KF[:], Sin,
                                         scale=imm["sin_scale"])
                else:
                    nc.scalar.activation(Q[:], S[:], Sin,
                                         scale=imm["sin_scale"])

            row0 = 0
            for bi, TG in enumerate(TREE_GROUPS):
                TB = sum(TG)            # samples in this tree group
                NF = TB // 2048         # 2048-sample leaf chunks
                trow0 = row0

                # leaves (biased, fp16) land in PSUM via PE transpose;
                # matmul blocks stay fine-grained for pipelining while the
                # tree tiles span the whole group (fewer, bigger DVE ops).
                LL = tpool.tile([128, NF, 2, 32], F16, name=f"LL_{bi}", tag="LL")
                S2 = tpool.tile([128, NF, 2, 32], F16, name=f"S2_{bi}", tag="S2")
                PB2 = tpool.tile([128, NF, 2, 32], F16, name=f"PB2_{bi}", tag="PB2")
                nfb = 0
                for bj, B in enumerate(TG):
                    R = B // 8              # xT columns
                    NP = B // 8192          # matmul pairs

                    xT = xpool.tile([128, R], F16, name=f"xT{bi}_{bj}",
                                    tag="xT")
                    half = R // 2
                    nc.sync.dma_start(out=xT[:, 0:half],
                                      in_=x_d[:, row0:row0 + half])
                    nc.sync.dma_start(out=xT[:, half:R],
                                      in_=x_d[:, row0 + half:row0 + R])

                    nq = (NP + 3) // 4
                    quarters = [(4 * qi, min(4 * qi + 4, NP))
                                for qi in range(nq)]
                    for hi, (p0, p1) in enumerate(quarters):
                        nfh = 4 * (p1 - p0)
                        leafT = lpool.tile([128, nfh, 128], F16,
                                           name=f"leafT{bi}_{bj}_{hi}",
                                           tag="leafT")
                        groups = [(c0, 2) for c0 in range(p0, p1 - 1, 2)]
                        if (p1 - p0) % 2:
                            groups.append((p1 - 1, 1))
                        for gi, (c0, ng) in enumerate(groups):
                            vps = ppool.tile([128, 1024], F32,
                                             name=f"vps{bi}_{bj}_{hi}_{gi}",
                                             tag="vps")
                            vt = vpool.tile([128, 1024], F16,
                                            name=f"vt{bi}_{bj}_{hi}_{gi}",
                                            tag="vt")
                            for q in range(2 * ng):
                                nc.tensor.matmul(
                                    vps[64 * (q % 2):64 * (q % 2) + 64,
                                        512 * (q // 2):512 * (q // 2) + 512],
                                    Gt,
                                    xT[:, 1024 * c0 + 512 * q:
                                       1024 * c0 + 512 * q + 512],
                                    start=True, stop=True)
                            nc.scalar.activation(vt[:, 0:512 * ng],
                                                 vps[:, 0:512 * ng],
                                                 Ident, bias=bvt[:, 0:1])
                            for u in range(4 * ng):
                                nc.tensor.transpose(
                                    leafT[:, 4 * (c0 - p0) + u, :],
                                    vt[:, 128 * u:128 * (u + 1)],
                                    idt)
                        lvh = leafT.rearrange("p n (h w) -> p n h w", h=2)
                        nf0 = nfb + 4 * p0
                        hsl = slice(nf0, nf0 + nfh)
                        nc.scalar.activation(LL[:, hsl], lvh[:, :, :, 0:32],
                                             Ident)
                        nc.vector.tensor_tensor(S2[:, hsl], LL[:, hsl],
                                                lvh[:, :, :, 32:64], ADD)
                        nc.vector.tensor_tensor(PB2[:, hsl], LL[:, hsl],
                                                lvh[:, :, :, 32:64], MUL)
                    nfb += B // 2048
                    row0 += R

                # ---- L2 ----
                imm = cc["L2"]
                Q2 = tpool.tile([128, NF, 2, 32], F16, name=f"Q2_{bi}", tag="Q2")
                K2 = tpool.tile([128, NF, 2, 32],
                                I32 if imm["k_i32"] else I16,
                                name=f"K2_{bi}", tag="K2")
                KF2 = tpool.tile([128, NF, 2, 32], F16, name=f"KF2_{bi}", tag="KF2")
                H2 = tpool.tile([128, NF, 2, 32], F16, name=f"H2_{bi}", tag="H2")
                rr_sin(S2, K2, KF2, Q2, imm, NF)
                for k in range(4):
                    sl = (slice(None), slice(None), slice(None),
                          slice(8 * k, 8 * k + 8))
                    nc.vector.tensor_scalar(H2[sl], S2[sl], imm["ts1"][k],
                                            imm["ts2"][k], MUL, ADD)
                    nc.vector.tensor_scalar(PB2[sl], PB2[sl],
                                            imm["sp"][k], None, MUL)
                    nc.vector.tensor_scalar(Q2[sl], Q2[sl], imm["sq"][k],
                                            None, MUL)
                split_tt(H2, H2, PB2, ADD, NF)
                split_tt(H2, H2, Q2, ADD, NF)

                # ---- L1 ----  (children at z 0:8 / 8:16 of kp groups)
                imm = cc["L1"]
                h2q = H2.rearrange("p n h (kp z) -> p (n h) kp z", kp=2)
                l1, r1 = h2q[:, :, :, 0:8], h2q[:, :, :, 8:16]
                S1 = tpool.tile([128, 2 * NF, 2, 8], F16, name=f"S1_{bi}", tag="S1")
                PB1 = tpool.tile([128, 2 * NF, 2, 8], F16, name=f"PB1_{bi}", tag="PB1")
                Q1 = tpool.tile([128, 2 * NF, 2, 8], F16, name=f"Q1_{bi}", tag="Q1")
                K1 = tpool.tile([128, 2 * NF, 2, 8],
                                I32 if imm["k_i32"] else I16,
                                name=f"K1_{bi}", tag="K1")
                KF1 = tpool.tile([128, 2 * NF, 2, 8], F16, name=f"KF1_{bi}", tag="KF1")
                H1 = tpool.tile([128, 2 * NF, 2, 8], F16, name=f"H1_{bi}", tag="H1")
                nc.vector.tensor_tensor(S1[:], l1, r1, ADD)
                nc.vector.tensor_tensor(PB1[:], l1, r1, MUL)
                rr_sin(S1, K1, KF1, Q1, imm, 2 * NF)
                for m in range(2):
                    sl = (slice(None), slice(None), slice(m, m + 1), slice(None))
                    nc.vector.tensor_scalar(H1[sl], S1[sl], imm["ts1"][m],
                                            imm["ts2"][m], MUL, ADD)
                    nc.vector.tensor_scalar(PB1[sl], PB1[sl],
                                            imm["sp"][m], None, MUL)
                for m in range(2):
                    sl = (slice(None), slice(None), slice(m, m + 1), slice(None))
                    nc.vector.tensor_scalar(Q1[sl], Q1[sl], imm["sq"][m],
                                            None, MUL)
                split_tt(H1, H1, PB1, ADD, 2 * NF)
                split_tt(H1, H1, Q1, ADD, 2 * NF)

                # ---- L0 ----
                imm = cc["L0"]
                S0 = tpool.tile([128, 2 * NF, 8], F16, name=f"S0_{bi}", tag="S0")
                PB0 = tpool.tile([128, 2 * NF, 8], F16, name=f"PB0_{bi}", tag="PB0")
                Q0 = tpool.tile([128, 2 * NF, 8], F16, name=f"Q0_{bi}", tag="Q0")
                K0 = tpool.tile([128, 2 * NF, 8],
                                I32 if imm["k_i32"] else I16,
                                name=f"K0_{bi}", tag="K0")
                KF0 = tpool.tile([128, 2 * NF, 8], F16, name=f"KF0_{bi}", tag="KF0")
                Y = ypool.tile([128, 2 * NF, 8], F16, name=f"Y_{bi}", tag="Y")
                nc.vector.tensor_tensor(S0[:], H1[:, :, 0:1, :], H1[:, :, 1:2, :], ADD)
                nc.vector.tensor_tensor(PB0[:], H1[:, :, 0:1, :],
                                        H1[:, :, 1:2, :], MUL)
                rr_sin(S0, K0, KF0, Q0, imm, 2 * NF)
                nc.vector.tensor_scalar(Y[:], S0[:], imm["ts1"][0],
                                        imm["ts2"][0], MUL, ADD)
                nc.vector.tensor_scalar(PB0[:], PB0[:], imm["sp"][0],
                                        None, MUL)
                nc.vector.tensor_scalar(Q0[:], Q0[:], imm["sq"][0], None, MUL)
                split_tt(Y, Y, PB0, ADD, 2 * NF)
                split_tt(Y, Y, Q0, ADD, 2 * NF)

                nc.sync.dma_start(out=y_d[:, trow0 // 16:
                                          trow0 // 16 + TB // 128],
                                  in_=Y[:])

    _split_excess_waits(nc)
    return nc


def _unpermute(y_core):
    """y_core [128, NPAD//128] fp16 -> [NPAD] f32 in sample order."""
    out = np.empty(NPAD, np.float32)
    base = 0
    col0 = 0
    for TG in TREE_GROUPS:
        TB = sum(TG)
        NP = TB // 8192
        FD = TB // 128
        yb = y_core[:, col0:col0 + FD].astype(np.float32)
        y5 = yb.reshape(128, NP, 4, 2, 8)          # q, c, u, h, j
        out[base:base + TB] = y5.transpose(1, 3, 2, 0, 4).reshape(TB)
        base += TB
        col0 += FD
    return out


def kernel(**inputs):
    x = np.asarray(inputs["x"], np.float32)
    xmax = float(np.abs(x).max())
    cc = _fold_params(inputs, xmax)

    nc = _build_program(cc)

    xp = np.zeros((N_CORES, NPAD, 16), np.float16)
    xp[:, :SHARD] = x.reshape(N_CORES, SHARD, 16).astype(np.float16)
    # [cores, 128, NPAD//8]: device layout xT[16j+v, r] = x[8r+j, v]
    xt = np.ascontiguousarray(
        xp.reshape(N_CORES, NPAD // 8, 128).transpose(0, 2, 1))

    c16 = np.concatenate([cc["G"], np.eye(128, dtype=np.float16)], axis=1)
    in_maps = [{"x": xt[c], "c16": c16, "bv": cc["biasvec"]}
               for c in range(N_CORES)]

    trace = bool(os.environ.get("BTREE_TRACE"))
    if trace:
        try:
            res = run_bass_kernel_spmd(nc, in_maps,
                                       core_ids=list(range(N_CORES)),
                                       trace=True)
        except Exception as e:
            print(f"trace run failed ({type(e).__name__}: {e}); rerunning untraced")
            res = run_bass_kernel_spmd(nc, in_maps,
                                       core_ids=list(range(N_CORES)))
    else:
        res = run_bass_kernel_spmd(nc, in_maps, core_ids=list(range(N_CORES)))
    globals()["LAST_RESULTS"] = res

    out = np.empty(N_TOTAL, np.float32)
    for c in range(N_CORES):
        yc = _unpermute(res.results[c]["y"])
        out[c * SHARD:(c + 1) * SHARD] = yc[:SHARD] * cc["g0"]
    return out


# revision 17
# speedup vs baseline: 1.0458x; 1.0017x over previous
"""BinaryTreeRNN forward pass on 8 Trainium2 NeuronCores.

Strategy (pure data parallel, per the sharding hint):
  - Shard x row-wise into 8 shards; replicate the ~100 tree parameters
    (folded into matmul weights + per-op float immediates on host).
  - Host pre-transposes x into the device layout [128, NPAD/8] fp16
    (partition = 16*j + v for 8 interleaved samples x 16 vars), so input
    DMA is plain contiguous at full HBM rate.
  - Per core: a block-diagonal [128, 64] fp16 stationary computes all 8
    leaves for 8 interleaved samples per PE column; leaves get bias during
    the PSUM->SBUF cast (ScalarE Identity + per-partition bias vector),
    then TensorE transposes put samples on partitions (PSUM fp16).
  - 3-level tree reduction, work split across engines:
      s = l + r (DVE TT 2x), p = l * r (DVE TT 2x)
      u = A*s + D (DVE TS 4x, per-node slices)
      p' = B*p (DVE TS 4x), h = u + p' (DVE TT)
      range-reduce: k = round(s*g/2pi) (DVE TS, int cast rounds),
        kf = -2pi/g*k (DVE TS 4x), r = s + kf (DVE TT 2x)
      q = sin(g*r)  (ScalarE); q' = C*q (DVE TS 4x)
      h += q' on Pool (TT); everything else on DVE
  - Power-of-two per-level scales keep fp16 intermediates in range.
  - Output written contiguously per device order, un-permuted on host.
"""
import os
import sys

sys.path.insert(0, "/opt/trn_rl_repo")

import numpy as np

import concourse.bass as bass
import concourse.mybir as mybir
import concourse.tile as tile
from concourse.bass_utils import run_bass_kernel_spmd

F16 = mybir.dt.float16
F32 = mybir.dt.float32
I16 = mybir.dt.int16
I32 = mybir.dt.int32

N_CORES = 8
N_TOTAL = 2_000_000
SHARD = N_TOTAL // N_CORES          # 250_000
TREE_GROUPS = [[8192, 16384], [65536, 16384], [65536, 8192], [65536, 8192]]
BLOCKS = [b for tg in TREE_GROUPS for b in tg]
NPAD = sum(BLOCKS)                  # 253_952
TWO_PI = float(2.0 * np.pi)

# leaf permutation: v' 0..3 = left children (leaves 0,2,4,6), 4..7 = right
PERM = np.array([0, 2, 4, 6, 1, 3, 5, 7])


def _sm(om):
    e = np.exp(om - om.max(axis=-1, keepdims=True))
    return e / e.sum(axis=-1, keepdims=True)


def _pow2_at_least(x):
    """Smallest power of two >= max(x, 1)."""
    return float(2.0 ** np.ceil(np.log2(max(float(x), 1.0))))


def _fold_params(inputs, xmax):
    """Fold tree parameters into device constants + per-op immediates."""
    W = np.asarray(inputs["W_leaf"], np.float64)
    bl = np.asarray(inputs["b_leaf"], np.float64)
    lv = {}
    for lev, nn in ((0, 1), (1, 2), (2, 4)):
        w = np.asarray(inputs[f"w{lev}"], np.float64)
        b = np.asarray(inputs[f"b{lev}"], np.float64)
        sm = _sm(np.asarray(inputs[f"om{lev}"], np.float64))
        lv[lev] = dict(
            A=w * (sm[:, 0] + sm[:, 3]),
            B=w * sm[:, 1],
            C=w * sm[:, 2],
            D=b,
        )

    # fp16 weights as actually used on device
    W16 = W[PERM].astype(np.float16).astype(np.float64)       # [8, 16] (perm order)
    bl16 = bl[PERM]                                            # bias kept fp32

    # interval bounds (true magnitudes)
    lb = (np.abs(W16).sum(axis=1) * xmax + np.abs(bl16)) * 1.05 + 1e-6  # [8]
    g_leaf = 1.0
    if lb.max() > 200.0:
        g_leaf = _pow2_at_least(lb.max() / 200.0)

    s2b = lb[0:4] + lb[4:8]
    p2b = lb[0:4] * lb[4:8]
    h2b = (np.abs(lv[2]["A"]) * s2b + np.abs(lv[2]["B"]) * p2b
           + np.abs(lv[2]["C"]) + np.abs(lv[2]["D"])) * 1.05 + 1e-6
    g2 = _pow2_at_least(h2b.max() / 200.0)

    s1b = h2b[0::2] + h2b[1::2]
    p1b = h2b[0::2] * h2b[1::2]
    h1b = (np.abs(lv[1]["A"]) * s1b + np.abs(lv[1]["B"]) * p1b
           + np.abs(lv[1]["C"]) + np.abs(lv[1]["D"])) * 1.05 + 1e-6
    g1 = _pow2_at_least(h1b.max() / 200.0)

    s0b = h1b[0] + h1b[1]
    p0b = h1b[0] * h1b[1]
    h0b = float(((np.abs(lv[0]["A"]) * s0b + np.abs(lv[0]["B"]) * p0b
                  + np.abs(lv[0]["C"]) + np.abs(lv[0]["D"])) * 1.05 + 1e-6)[0])
    g0 = _pow2_at_least(h0b / 40000.0)

    # blockdiag stationary G [128, 64] (leaf scale folded in)
    G = np.zeros((128, 64), np.float32)
    for j in range(8):
        for vp in range(8):
            G[16 * j:16 * j + 16, 8 * vp + j] = (W16[vp] / g_leaf).astype(np.float32)
    biasvec = np.zeros((128, 1), np.float32)
    for h in range(2):
        for vp in range(8):
            for j in range(8):
                biasvec[64 * h + 8 * vp + j, 0] = bl16[vp] / g_leaf

    # per-level op immediates: children scale gp -> own scale gc
    s_bounds = {2: float(s2b.max()), 1: float(s1b.max()), 0: float(s0b)}

    def imm(lev, gp, gc):
        d = lv[lev]
        kmax = s_bounds[lev] / (2.0 * np.pi)   # |k| bound (true periods)
        return dict(
            ts1=[float(a * gp / gc) for a in d["A"]],
            ts2=[float(dd / gc) for dd in d["D"]],
            sp=[float(b * gp * gp / gc) for b in d["B"]],
            sq=[float(c / gc) for c in d["C"]],
            k_scale=float(gp / TWO_PI),
            kf_scale=float(-TWO_PI / gp),
            sin_scale=float(gp),
            k_i32=bool(kmax > 30000.0),
            need_rr=bool(s_bounds[lev] > 3.0),
        )

    return dict(
        G=G.astype(np.float16),
        biasvec=biasvec,
        L2=imm(2, g_leaf, g2),
        L1=imm(1, g2, g1),
        L0=imm(0, g1, g0),
        g0=float(g0),
    )


# ---------------------------------------------------------------------------
# walrus in this container accepts at most ONE sync-wait per instruction
# (2 for InstEventSemaphore); hoist excess waits onto InstNoOp carriers.
def _split_excess_waits(nc):
    n_fix = 0
    for fn in nc.m.functions:
        for blk in fn.blocks:
            new_insts = []
            for inst in blk.instructions:
                si = inst.sync_info
                cap = 2 if isinstance(inst, mybir.InstEventSemaphore) else 1
                if si is not None and len(si.on_wait) > cap:
                    waits = list(si.on_wait)
                    for w in waits[:-cap]:
                        new_insts.append(mybir.InstNoOp(
                            name=f"{inst.name}-waitc{n_fix}",
                            ins=[], outs=[],
                            sync_info=mybir.SyncInfo(on_wait=[w], on_update=[]),
                            bass_nofuse=True,
                            engine=inst.engine,
                        ))
                        n_fix += 1
                    inst.sync_info = mybir.SyncInfo(
                        on_wait=waits[-cap:], on_update=list(si.on_update))
                new_insts.append(inst)
            blk.instructions[:] = new_insts
    return n_fix


def _build_program(cc):
    """cc: folded constants (for the float immediates)."""
    nc = bass.Bass("TRN2", target_bir_lowering=False, debug=False,
                   num_devices=N_CORES)
    x_d = nc.dram_tensor("x", [128, NPAD // 8], F16, kind="ExternalInput").ap()
    c16_d = nc.dram_tensor("c16", [128, 192], F16, kind="ExternalInput").ap()
    bv_d = nc.dram_tensor("bv", [128, 1], F32, kind="ExternalInput").ap()
    y_d = nc.dram_tensor("y", [128, NPAD // 128], F16, kind="ExternalOutput").ap()

    Sin = mybir.ActivationFunctionType.Sin
    Ident = mybir.ActivationFunctionType.Identity
    MUL = mybir.AluOpType.mult
    ADD = mybir.AluOpType.add
    MOD = mybir.AluOpType.mod

    with tile.TileContext(nc) as tc:
        with tc.tile_pool(name="cpool", bufs=1) as cpool, \
             tc.tile_pool(name="xpool", bufs=2) as xpool, \
             tc.tile_pool(name="vpool", bufs=4) as vpool, \
             tc.tile_pool(name="tpool", bufs=1) as tpool, \
             tc.tile_pool(name="ypool", bufs=2) as ypool, \
             tc.tile_pool(name="psum", bufs=2, space="PSUM") as ppool, \
             tc.tile_pool(name="psumL", bufs=2, space="PSUM") as lpool:

            c16 = cpool.tile([128, 192], F16)
            bvt = cpool.tile([128, 1], F32)
            with tc.high_priority():
                nc.sync.dma_start(out=c16[:], in_=c16_d[:])
                nc.sync.dma_start(out=bvt[:], in_=bv_d[:])
            Gt = c16[:, 0:64]
            idt = c16[:, 64:192]

            def split_tt(out, a, b, op, nf, pool_frac=0.34):
                """TT with the free range split DVE/Pool so both engines run
                in parallel on disjoint slices (dim 1 = NF chunks)."""
                k = max(1, min(nf - 1, int(round(nf * (1.0 - pool_frac)))))
                nc.vector.tensor_tensor(out[:, 0:k], a[:, 0:k], b[:, 0:k], op)
                nc.gpsimd.tensor_tensor(out[:, k:nf], a[:, k:nf], b[:, k:nf],
                                        op)

            def rr_sin(S, Kt, KF, Q, imm, nf):
                """Q = sin(gp*s): k = round(s*gp/2pi) (DVE int cast rounds),
                kf = -2pi/gp * k, r = s + kf, sin(gp*r) on ScalarE."""
                if imm["need_rr"]:
                    nc.vector.tensor_scalar(Kt[:], S[:], imm["k_scale"],
                                            None, MUL)
                    nc.vector.tensor_scalar(KF[:], Kt[:], imm["kf_scale"],
                                            None, MUL)
                    split_tt(KF, S, KF, ADD, nf)
                    nc.scalar.activation(Q[:], KF[:], Sin,
                                         scale=imm["sin_scale"])
                else:
                    nc.scalar.activation(Q[:], S[:], Sin,
                                         scale=imm["sin_scale"])

            # --- software-pipelined emission -------------------------
            # Engines execute in order per queue; to keep DVE fed during a
            # group's sin waits, the NEXT group's leaf blocks are emitted
            # interleaved with this group's tree stages.
            gctx = {}

            def alloc_group(bi, TG, trow0):
                TB = sum(TG)
                NF = TB // 2048
                gctx[bi] = dict(
                    TB=TB, NF=NF, trow0=trow0,
                    LL=tpool.tile([128, NF, 2, 32], F16, name=f"LL_{bi}",
                                  tag="LL"),
                    S2=tpool.tile([128, NF, 2, 32], F16, name=f"S2_{bi}",
                                  tag="S2"),
                    PB2=tpool.tile([128, NF, 2, 32], F16, name=f"PB2_{bi}",
                                   tag="PB2"),
                )

            def emit_leaf_block(bi, bj, B, xrow, nfb):
                g = gctx[bi]
                LL, S2, PB2 = g["LL"], g["S2"], g["PB2"]
                R = B // 8
                NP = B // 8192
                xT = xpool.tile([128, R], F16, name=f"xT{bi}_{bj}", tag="xT")
                half = R // 2
                nc.sync.dma_start(out=xT[:, 0:half],
                                  in_=x_d[:, xrow:xrow + half])
                nc.sync.dma_start(out=xT[:, half:R],
                                  in_=x_d[:, xrow + half:xrow + R])
                nq = (NP + 3) // 4
                quarters = [(4 * qi, min(4 * qi + 4, NP)) for qi in range(nq)]
                for hi, (p0, p1) in enumerate(quarters):
                    nfh = 4 * (p1 - p0)
                    leafT = lpool.tile([128, nfh, 128], F16,
                                       name=f"leafT{bi}_{bj}_{hi}",
                                       tag="leafT")
                    groups = [(c0, 2) for c0 in range(p0, p1 - 1, 2)]
                    if (p1 - p0) % 2:
                        groups.append((p1 - 1, 1))
                    for gi, (c0, ng) in enumerate(groups):
                        vps = ppool.tile([128, 1024], F32,
                                         name=f"vps{bi}_{bj}_{hi}_{gi}",
                                         tag="vps")
                        vt = vpool.tile([128, 1024], F16,
                                        name=f"vt{bi}_{bj}_{hi}_{gi}",
                                        tag="vt")
                        for q in range(2 * ng):
                            nc.tensor.matmul(
                                vps[64 * (q % 2):64 * (q % 2) + 64,
                                    512 * (q // 2):512 * (q // 2) + 512],
                                Gt,
                                xT[:, 1024 * c0 + 512 * q:
                                   1024 * c0 + 512 * q + 512],
                                start=True, stop=True)
                        nc.scalar.activation(vt[:, 0:512 * ng],
                                             vps[:, 0:512 * ng],
                                             Ident, bias=bvt[:, 0:1])
                        for u in range(4 * ng):
                            nc.tensor.transpose(
                                leafT[:, 4 * (c0 - p0) + u, :],
                                vt[:, 128 * u:128 * (u + 1)],
                                idt)
                    lvh = leafT.rearrange("p n (h w) -> p n h w", h=2)
                    nf0 = nfb + 4 * p0
                    hsl = slice(nf0, nf0 + nfh)
                    nc.scalar.activation(LL[:, hsl], lvh[:, :, :, 0:32],
                                         Ident)
                    nc.vector.tensor_tensor(S2[:, hsl], LL[:, hsl],
                                            lvh[:, :, :, 32:64], ADD)
                    nc.vector.tensor_tensor(PB2[:, hsl], LL[:, hsl],
                                            lvh[:, :, :, 32:64], MUL)

            def stage_a(bi):
                g = gctx[bi]
                NF = g["NF"]
                S2, PB2 = g["S2"], g["PB2"]
                imm = cc["L2"]
                Q2 = tpool.tile([128, NF, 2, 32], F16, name=f"Q2_{bi}",
                                tag="Q2")
                K2 = tpool.tile([128, NF, 2, 32],
                                I32 if imm["k_i32"] else I16,
                                name=f"K2_{bi}", tag="K2")
                KF2 = tpool.tile([128, NF, 2, 32], F16, name=f"KF2_{bi}",
                                 tag="KF2")
                H2 = tpool.tile([128, NF, 2, 32], F16, name=f"H2_{bi}",
                                tag="H2")
                g.update(Q2=Q2, K2=K2, KF2=KF2, H2=H2)
                rr_sin(S2, K2, KF2, Q2, imm, NF)
                for k in range(4):
                    sl = (slice(None), slice(None), slice(None),
                          slice(8 * k, 8 * k + 8))
                    nc.vector.tensor_scalar(H2[sl], S2[sl], imm["ts1"][k],
                                            imm["ts2"][k], MUL, ADD)
                    nc.vector.tensor_scalar(PB2[sl], PB2[sl],
                                            imm["sp"][k], None, MUL)

            def stage_b(bi):
                g = gctx[bi]
                NF = g["NF"]
                Q2, H2, PB2 = g["Q2"], g["H2"], g["PB2"]
                imm = cc["L2"]
                for k in range(4):
                    sl = (slice(None), slice(None), slice(None),
                          slice(8 * k, 8 * k + 8))
                    nc.vector.tensor_scalar(Q2[sl], Q2[sl], imm["sq"][k],
                                            None, MUL)
                split_tt(H2, H2, PB2, ADD, NF)
                split_tt(H2, H2, Q2, ADD, NF)

                # ---- L1 pre-sin ----
                imm = cc["L1"]
                h2q = H2.rearrange("p n h (kp z) -> p (n h) kp z", kp=2)
                l1, r1 = h2q[:, :, :, 0:8], h2q[:, :, :, 8:16]
                S1 = tpool.tile([128, 2 * NF, 2, 8], F16, name=f"S1_{bi}",
                                tag="S1")
                PB1 = tpool.tile([128, 2 * NF, 2, 8], F16, name=f"PB1_{bi}",
                                 tag="PB1")
                Q1 = tpool.tile([128, 2 * NF, 2, 8], F16, name=f"Q1_{bi}",
                                tag="Q1")
                K1 = tpool.tile([128, 2 * NF, 2, 8],
                                I32 if imm["k_i32"] else I16,
                                name=f"K1_{bi}", tag="K1")
                KF1 = tpool.tile([128, 2 * NF, 2, 8], F16, name=f"KF1_{bi}",
                                 tag="KF1")
                H1 = tpool.tile([128, 2 * NF, 2, 8], F16, name=f"H1_{bi}",
                                tag="H1")
                g.update(S1=S1, PB1=PB1, Q1=Q1, H1=H1)
                nc.vector.tensor_tensor(S1[:], l1, r1, ADD)
                nc.vector.tensor_tensor(PB1[:], l1, r1, MUL)
                rr_sin(S1, K1, KF1, Q1, imm, 2 * NF)
                for m in range(2):
                    sl = (slice(None), slice(None), slice(m, m + 1),
                          slice(None))
                    nc.vector.tensor_scalar(H1[sl], S1[sl], imm["ts1"][m],
                                            imm["ts2"][m], MUL, ADD)
                    nc.vector.tensor_scalar(PB1[sl], PB1[sl],
                                            imm["sp"][m], None, MUL)

            def stage_c(bi):
                g = gctx[bi]
                NF = g["NF"]
                Q1, H1, PB1 = g["Q1"], g["H1"], g["PB1"]
                imm = cc["L1"]
                for m in range(2):
                    sl = (slice(None), slice(None), slice(m, m + 1),
                          slice(None))
                    nc.vector.tensor_scalar(Q1[sl], Q1[sl], imm["sq"][m],
                                            None, MUL)
                split_tt(H1, H1, PB1, ADD, 2 * NF)
                split_tt(H1, H1, Q1, ADD, 2 * NF)

                # ---- L0 pre-sin ----
                imm = cc["L0"]
                S0 = tpool.tile([128, 2 * NF, 8], F16, name=f"S0_{bi}",
                                tag="S0")
                PB0 = tpool.tile([128, 2 * NF, 8], F16, name=f"PB0_{bi}",
                                 tag="PB0")
                Q0 = tpool.tile([128, 2 * NF, 8], F16, name=f"Q0_{bi}",
                                tag="Q0")
                K0 = tpool.tile([128, 2 * NF, 8],
                                I32 if imm["k_i32"] else I16,
                                name=f"K0_{bi}", tag="K0")
                KF0 = tpool.tile([128, 2 * NF, 8], F16, name=f"KF0_{bi}",
                                 tag="KF0")
                Y = ypool.tile([128, 2 * NF, 8], F16, name=f"Y_{bi}", tag="Y")
                g.update(S0=S0, PB0=PB0, Q0=Q0, Y=Y)
                nc.vector.tensor_tensor(S0[:], H1[:, :, 0:1, :],
                                        H1[:, :, 1:2, :], ADD)
                nc.vector.tensor_tensor(PB0[:], H1[:, :, 0:1, :],
                                        H1[:, :, 1:2, :], MUL)
                rr_sin(S0, K0, KF0, Q0, imm, 2 * NF)
                nc.vector.tensor_scalar(Y[:], S0[:], imm["ts1"][0],
                                        imm["ts2"][0], MUL, ADD)
                nc.vector.tensor_scalar(PB0[:], PB0[:], imm["sp"][0],
                                        None, MUL)

            def stage_d(bi):
                g = gctx[bi]
                NF = g["NF"]
                Q0, PB0, Y = g["Q0"], g["PB0"], g["Y"]
                imm = cc["L0"]
                nc.vector.tensor_scalar(Q0[:], Q0[:], imm["sq"][0], None, MUL)
                split_tt(Y, Y, PB0, ADD, 2 * NF)
                split_tt(Y, Y, Q0, ADD, 2 * NF)
                trow0 = g["trow0"]
                nc.sync.dma_start(out=y_d[:, trow0 // 16:
                                          trow0 // 16 + g["TB"] // 128],
                                  in_=Y[:])

            # group geometry
            geo = []
            xrow = 0
            for bi, TG in enumerate(TREE_GROUPS):
                blocks = []
                nfb = 0
                trow0 = xrow
                for bj, B in enumerate(TG):
                    blocks.append((bj, B, xrow, nfb))
                    xrow += B // 8
                    nfb += B // 2048
                geo.append((TG, trow0, blocks))

            def emit_group_leaves(bi):
                TG, trow0, blocks = geo[bi]
                alloc_group(bi, TG, trow0)
                return [(bi, bj, B, xr, nfb) for (bj, B, xr, nfb) in blocks]

            nG = len(TREE_GROUPS)
            pend = list(emit_group_leaves(0))
            for bj, B, xr, nfb in [p[1:] for p in pend]:
                emit_leaf_block(0, bj, B, xr, nfb)
            for bi in range(nG):
                nxt = emit_group_leaves(bi + 1) if bi + 1 < nG else []
                stages = [stage_a, stage_b, stage_c, stage_d]
                qi = 0
                for si, st in enumerate(stages):
                    st(bi)
                    if qi < len(nxt) and si < 3:
                        a, bj, B, xr, nfb = nxt[qi]
                        emit_leaf_block(a, bj, B, xr, nfb)
                        qi += 1
                while qi < len(nxt):
                    a, bj, B, xr, nfb = nxt[qi]
                    emit_leaf_block(a, bj, B, xr, nfb)
                    qi += 1

    _split_excess_waits(nc)
    return nc


def _unpermute(y_core):
    """y_core [128, NPAD//128] fp16 -> [NPAD] f32 in sample order."""
    out = np.empty(NPAD, np.float32)
    base = 0
    col0 = 0
    for TG in TREE_GROUPS:
        TB = sum(TG)
        NP = TB // 8192
        FD = TB // 128
        yb = y_core[:, col0:col0 + FD].astype(np.float32)
        y5 = yb.reshape(128, NP, 4, 2, 8)          # q, c, u, h, j
        out[base:base + TB] = y5.transpose(1, 3, 2, 0, 4).reshape(TB)
        base += TB
        col0 += FD
    return out


def kernel(**inputs):
    x = np.asarray(inputs["x"], np.float32)
    xmax = float(np.abs(x).max())
    cc = _fold_params(inputs, xmax)

    nc = _build_program(cc)

    xp = np.zeros((N_CORES, NPAD, 16), np.float16)
    xp[:, :SHARD] = x.reshape(N_CORES, SHARD, 16).astype(np.float16)
    # [cores, 128, NPAD//8]: device layout xT[16j+v, r] = x[8r+j, v]
    xt = np.ascontiguousarray(
        xp.reshape(N_CORES, NPAD // 8, 128).transpose(0, 2, 1))

    c16 = np.concatenate([cc["G"], np.eye(128, dtype=np.float16)], axis=1)
    in_maps = [{"x": xt[c], "c16": c16, "bv": cc["biasvec"]}
               for c in range(N_CORES)]

    trace = bool(os.environ.get("BTREE_TRACE"))
    if trace:
        try:
            res = run_bass_kernel_spmd(nc, in_maps,
                                       core_ids=list(range(N_CORES)),
                                       trace=True)
        except Exception as e:
            print(f"trace run failed ({type(e).__name__}: {e}); rerunning untraced")
            res = run_bass_kernel_spmd(nc, in_maps,
                                       core_ids=list(range(N_CORES)))
    else:
        res = run_bass_kernel_spmd(nc, in_maps, core_ids=list(range(N_CORES)))
    globals()["LAST_RESULTS"] = res

    out = np.empty(N_TOTAL, np.float32)
    for c in range(N_CORES):
        yc = _unpermute(res.results[c]["y"])
        out[c * SHARD:(c + 1) * SHARD] = yc[:SHARD] * cc["g0"]
    return out
